# revision 11
# baseline (speedup 1.0000x reference)
"""CTC loss kernel for Trainium2 (8 NeuronCores, data-parallel over batch).

Strategy
--------
Per core: 64 examples. The CTC forward DP runs in probability space with
states in the free dim and (example, direction) packed into the 128
partitions: rows 0-63 run the forward DP for t=0..255, rows 64-127 run
the suffix (backward) DP in state-reversed coordinates for t=511..256.
The two halves are spliced at T/2 on host: P = sum_s alpha_255[s]*W_255[s].

Emissions are produced on the TensorEngine: transpose y_pred (bf16) to
(class, t) then a one-hot matmul (N=132). The first two 128-t blocks
are MIXED (64 fwd t-steps in partitions 0-63, 64 bwd t-steps in 64-127,
one N=264 [fwd|bwd] gather, quadrant ACT copies) so the DP can start
after one block. A DRAM round-trip reshuffles (t-part, ex, 132) into
(example-part, tau-major) chunks, landing in the ec half of an
interleaved [ecm(132)|ec(132)] per-tau layout; a bulk DVE multiply by
the replicated skip mask fills ecm = m2*ec.

The DP is split into two independent interleaved strands so every DVE
dependency is >= 2 instructions back (same-engine semaphore waits
resolve for free): strand A owns mirrored states [0:66) and carries a
32-col ghost ([0:98) computed), strand B owns [66:132). The per-step
information creep (2 cols/step) eats the ghost; every RESC=16 steps the
ghost is refreshed from B and the rows rescaled to a 2^43 setpoint.
Each strand step is 3 ops on a packed tile T = [alpha | guard(2) | u]:
  u = alpha[0:w] + alpha[1:w+1];  W = T[2:2+2w] * [ecm|ec](strided);
  alpha' = W[0:w] + W[w:2w]
States are MIRRORED (guards at top) so the information flow is upward-
only: B needs no ghost from A.

Numerics: bf16 state, approx-reciprocal rescale (max history out, logs
added on host), no eps bias (shifts loss ~1e-5 relative), final splice
on host in f64.
"""

import numpy as np

B, T, C, L = 512, 512, 96, 64
BLANK = C - 1
S = 2 * L + 1          # 129 states
SW = 132               # padded state width (multiple of 4)
EW = 2 * SW            # per-tau DP emission width: [ecm | ec]
NCORES = 8
BN = B // NCORES       # 64 examples per core
TH = T // 2            # split point
CHK = 32               # tau chunk size
RESC = 16              # rescale + ghost-refresh period
NRESC = (TH - 1) // RESC  # 15 rescales
SETPOINT_LOG2 = 43     # rescale normalizes row max to 2^43
G = 60.646622          # exp(mean_loss/T) boost; keeps alpha ~O(1) per step
SA = 66                # strand A owns mirrored states [0:SA)
GH = 2 * RESC          # ghost width (creep budget between refreshes)
WA = SA + GH           # strand A computed width (98)
WB = SW - SA           # strand B width (66)

_BUILT = None
_LAST_EXEC_NS = None
_LAST_RES = None


def _host_metadata(y_true):
    """ext labels, skip masks, init masks, per-state classes — from y_true."""
    y_true = np.asarray(y_true, dtype=np.int32)
    lbl_len = (y_true != -1).sum(axis=-1).astype(np.int32)
    labels = np.where(y_true != -1, y_true, 0).astype(np.int32)
    ext = np.full((B, S), BLANK, np.int32)
    ext[:, 1::2] = labels
    ext_m2 = np.pad(ext[:, :-2], ((0, 0), (2, 0)), constant_values=BLANK)
    can_skip = ((ext != BLANK) & (ext != ext_m2)).astype(np.float32)

    m2f = np.zeros((B, SW), np.float32)
    m2f[:, :S] = can_skip
    m2b = np.zeros((B, SW), np.float32)
    for u in range(2, S):
        m2b[:, u] = can_skip[:, S - 1 - u + 2]

    mif = np.zeros((B, SW), np.float32)
    mif[:, 0] = 1.0
    mif[:, 1] = 1.0
    mib = np.zeros((B, SW), np.float32)
    mib[np.arange(B), S - 1 - 2 * lbl_len] = 1.0
    mib[np.arange(B), S - 1 - (2 * lbl_len - 1)] = 1.0

    clsf = np.full((B, SW), -1, np.int32)           # -1 -> all-zero column
    clsf[:, :S] = ext
    clsb = np.full((B, SW), -1, np.int32)
    clsb[:, :S] = ext[:, ::-1]
    return m2f, m2b, mif, mib, clsf, clsb


def _build(num_cores=NCORES, t_full=T, bn=BN):
    """Build and schedule the Bass module once."""
    import concourse.bacc as bacc
    import concourse.mybir as mybir
    import concourse.tile as tile
    from contextlib import ExitStack
    from concourse.vector_clock import ScopedClock

    def _patched_drain_and_barrier(self, tick_clock, wait_clock):
        nc = self.nc
        drain_inst = nc.sync.drain()
        wait_clock.add_sem_waits(
            drain_inst.ins, ScopedClock({None: tick_clock.global_clock})
        )
        si = drain_inst.ins.sync_info
        waits = list(si.on_wait) if si and si.on_wait else []
        if len(waits) > 1:
            si.on_wait = waits[:1]
            for w in waits[1:]:
                extra = nc.sync.drain()
                esi = extra.ins.sync_info
                if esi is None:
                    extra.ins.sync_info = mybir.SyncInfo(on_wait=[w], on_update=[])
                else:
                    esi.on_wait = (esi.on_wait or []) + [w]
        nc.all_engine_barrier()
        assert self.sems is not None
        popped = nc._tile_sem_poison_stack.pop()
        assert popped is self._sem_poison
        nc.clear_and_free_semaphores(list(self.sems.allocated().values()))
        nc.all_engine_barrier()

    tile.TileContext._drain_and_barrier = _patched_drain_and_barrier

    f32 = mybir.dt.float32
    bf16 = mybir.dt.bfloat16
    AX = mybir.AxisListType.X
    COPY = mybir.ActivationFunctionType.Copy
    MULT = mybir.AluOpType.mult
    MAXO = mybir.AluOpType.max

    th = t_full // 2
    chk = CHK
    nresc = (th - 1) // RESC

    nc = bacc.Bacc("TRN2", target_bir_lowering=False, debug=False,
                   num_devices=num_cores)
    ypred = nc.dram_tensor("ypred", [bn, t_full, C], f32, kind="ExternalInput")
    m2_in = nc.dram_tensor("m2", [128, SW], bf16, kind="ExternalInput")
    m2r_in = nc.dram_tensor("m2rep", [128, chk * SW], bf16, kind="ExternalInput")
    mi_in = nc.dram_tensor("minit", [128, SW], bf16, kind="ExternalInput")
    oh_in = nc.dram_tensor("onehot", [C, bn * 2 * SW], bf16, kind="ExternalInput")
    id_in = nc.dram_tensor("ident", [128, 128], bf16, kind="ExternalInput")
    h_out = nc.dram_tensor("hist", [128, max(nresc, 1)], f32, kind="ExternalOutput")
    a_out = nc.dram_tensor("afin", [128, SW + 2], bf16, kind="ExternalOutput")
    g_out = nc.dram_tensor("gfin", [128, SW], bf16, kind="ExternalOutput")

    with tile.TileContext(nc) as tc, ExitStack() as ctx:
        const = ctx.enter_context(tc.tile_pool(name="const", bufs=1))
        dramp = ctx.enter_context(tc.tile_pool(name="edram", bufs=1, space="DRAM"))
        ypf_pool = ctx.enter_context(tc.tile_pool(name="ypf", bufs=2))
        ypb_pool = ctx.enter_context(tc.tile_pool(name="ypb", bufs=2))
        stg_pool = ctx.enter_context(tc.tile_pool(name="stg", bufs=4))
        ec_pool = ctx.enter_context(tc.tile_pool(name="ec", bufs=3))
        ytp_pool = ctx.enter_context(tc.tile_pool(name="ytp", bufs=2, space="PSUM"))
        eps_pool = ctx.enter_context(tc.tile_pool(name="eps", bufs=4, space="PSUM"))
        yts_pool = ctx.enter_context(tc.tile_pool(name="yts", bufs=2))

        m2t = const.tile([128, SW], bf16, tag="m2t")
        m2rep = const.tile([128, chk * SW], bf16, tag="m2rep")
        mit = const.tile([128, SW], bf16, tag="mit")
        oht = const.tile([C, bn * 2 * SW], bf16, tag="oht")
        identt = const.tile([128, 128], bf16, tag="identt")
        TA = const.tile([128, 2 * WA + 2], bf16, tag="TA")   # [alphaA|g2|uA]
        TB = const.tile([128, 2 * WB + 2], bf16, tag="TB")   # [alphaB|g2|uB]
        WAt = const.tile([128, 2 * WA], bf16, tag="WAt")
        WBt = const.tile([128, 2 * WB], bf16, tag="WBt")
        alf = const.tile([128, SW + 2], bf16, tag="alf")
        ut = const.tile([128, SW], bf16, tag="ut")
        vt = const.tile([128, SW], bf16, tag="vt")
        wt = const.tile([128, SW], bf16, tag="wt")
        histt = const.tile([128, max(nresc, 1)], f32, tag="histt")
        hB = const.tile([128, 1], f32, tag="hB")
        sclt = const.tile([128, 1], f32, tag="sclt")

        nc.sync.dma_start(out=m2t[:, :], in_=m2_in.ap())
        nc.sync.dma_start(out=m2rep[:, :], in_=m2r_in.ap())
        nc.sync.dma_start(out=mit[:, :], in_=mi_in.ap())
        nc.sync.dma_start(out=identt[:, :], in_=id_in.ap())
        gw = 16 * 2 * SW
        for g in range(bn // 16):
            nc.sync.dma_start(out=oht[:, g * gw:(g + 1) * gw],
                              in_=oh_in.ap()[:, g * gw:(g + 1) * gw])
        nc.vector.memset(histt[:, :], 0.0)
        nc.vector.memset(TA[:, :], 0.0)
        nc.vector.memset(TB[:, :], 0.0)
        nc.vector.memset(alf[:, :], 0.0)

        # ---- phase A + B: emissions and chunk shuffle ----
        # blocks: two MIXED (fwd 64t | bwd 64t) then two normal 128-t
        blocks = [("mix", 0, 0, 448), ("mix", 1, 64, 384),
                  ("nrm", 2, 1, 0), ("nrm", 3, 2, 1)]
        ecc_tiles = {}
        for bi, (kind, tagn, p0, p1) in enumerate(blocks):
            ed = dramp.tile([128, bn * SW], bf16, tag=f"ed{tagn}")
            early = bi < 2
            for grp in range(bn // 16):
                ypf = ypf_pool.tile([128, 16 * C], f32)
                yp3 = ypf[:, :].rearrange("p (e c) -> p e c", c=C)
                if kind == "mix":
                    f0, b0 = p0, p1
                    srcf = ypred.ap()[grp * 16:(grp + 1) * 16, f0:f0 + 64, :]
                    srcb = ypred.ap()[grp * 16:(grp + 1) * 16, b0:b0 + 64, :]
                    nc.sync.dma_start(out=yp3[0:64, :, :],
                                      in_=srcf.rearrange("e t c -> t e c"))
                    nc.sync.dma_start(out=yp3[64:128, :, :],
                                      in_=srcb.rearrange("e t c -> t e c"))
                else:
                    k = p0
                    src = ypred.ap()[grp * 16:(grp + 1) * 16,
                                     k * 128:(k + 1) * 128, :]
                    nc.sync.dma_start(out=yp3[:, :, :],
                                      in_=src.rearrange("e t c -> t e c"))
                ypb = ypb_pool.tile([128, 16 * C], bf16)
                if early:
                    nc.vector.tensor_copy(ypb[:, :], ypf[:, :])
                else:
                    nc.gpsimd.tensor_copy(ypb[:, :], ypf[:, :])
                for q in range(4):
                    ytp = ytp_pool.tile([C, 512], bf16)
                    for e4 in range(4):
                        e = q * 4 + e4
                        nc.tensor.transpose(
                            ytp[:, e4 * 128:(e4 + 1) * 128],
                            ypb[:, e * C:(e + 1) * C],
                            identt[:, :])
                    yts = yts_pool.tile([C, 512], bf16)
                    nc.scalar.activation(yts[:, :], ytp[:, :], COPY,
                                         bias=0.0, scale=1.0)
                    stg = stg_pool.tile([128, 4 * SW], bf16)
                    for e4 in range(4):
                        e = q * 4 + e4
                        ex = grp * 16 + e
                        if kind == "mix":
                            # one N=264 [fwd|bwd] gather, quadrant copies
                            epsum = eps_pool.tile([128, EW], f32)
                            nc.tensor.matmul(
                                epsum[:, :],
                                yts[:, e4 * 128:(e4 + 1) * 128],
                                oht[:, ex * 2 * SW:ex * 2 * SW + EW],
                                start=True, stop=True)
                            nc.scalar.activation(
                                stg[0:64, e4 * SW:(e4 + 1) * SW],
                                epsum[0:64, 0:SW],
                                COPY, bias=0.0, scale=float(G))
                            nc.scalar.activation(
                                stg[64:128, e4 * SW:(e4 + 1) * SW],
                                epsum[64:128, SW:EW],
                                COPY, bias=0.0, scale=float(G))
                        else:
                            dirn = p1
                            ohoff = (ex * 2 + dirn) * SW
                            epsum = eps_pool.tile([128, SW], f32)
                            nc.tensor.matmul(
                                epsum[:, :],
                                yts[:, e4 * 128:(e4 + 1) * 128],
                                oht[:, ohoff:ohoff + SW],
                                start=True, stop=True)
                            nc.scalar.activation(
                                stg[:, e4 * SW:(e4 + 1) * SW], epsum[:, :],
                                COPY, bias=0.0, scale=float(G))
                    ex0 = grp * 16 + q * 4
                    nc.sync.dma_start(
                        out=ed[:, ex0 * SW:(ex0 + 4) * SW], in_=stg[:, :])

            # chunk emission
            if kind == "mix":
                m = tagn
                for jj in range(2):
                    j = 2 * m + jj
                    ecc = ec_pool.tile([128, chk * EW], bf16)
                    ec3 = ecc[:, :].rearrange("p (t s) -> p t s", s=EW)
                    src3 = ed[:, :].rearrange("t (e s) -> t e s", s=SW)
                    fr0 = jj * chk
                    br0 = 127 - jj * chk
                    bsl = (slice(br0, None, -1) if br0 - chk < 0
                           else slice(br0, br0 - chk, -1))
                    nc.sync.dma_start(
                        out=ec3[0:64, :, SW:EW],
                        in_=src3[fr0:fr0 + chk, :, :].rearrange("t e s -> e t s"))
                    nc.sync.dma_start(
                        out=ec3[64:128, :, SW:EW],
                        in_=src3[bsl, :, :].rearrange("t e s -> e t s"))
                    nc.vector.tensor_mul(
                        ec3[:, :, 0:SW], ec3[:, :, SW:EW],
                        m2rep[:, :].rearrange("p (t s) -> p t s", s=SW))
                    ecc_tiles[j] = ecc
            elif tagn == 3:     # after both normal blocks: chunks 4-7
                for jj in range(4):
                    j = 4 + jj
                    ecc = ec_pool.tile([128, chk * EW], bf16)
                    ec3 = ecc[:, :].rearrange("p (t s) -> p t s", s=EW)
                    # fwd from block k=1 (tag 2), bwd from block k=2 (tag 3)
                    f3 = ed_nrm1[:, :].rearrange("t (e s) -> t e s", s=SW)
                    b3 = ed[:, :].rearrange("t (e s) -> t e s", s=SW)
                    tl0 = jj * chk
                    tb0 = 127 - jj * chk
                    bsl = (slice(tb0, None, -1) if tb0 - chk < 0
                           else slice(tb0, tb0 - chk, -1))
                    nc.sync.dma_start(
                        out=ec3[0:64, :, SW:EW],
                        in_=f3[tl0:tl0 + chk, :, :].rearrange("t e s -> e t s"))
                    nc.sync.dma_start(
                        out=ec3[64:128, :, SW:EW],
                        in_=b3[bsl, :, :].rearrange("t e s -> e t s"))
                    nc.vector.tensor_mul(
                        ec3[:, :, 0:SW], ec3[:, :, SW:EW],
                        m2rep[:, :].rearrange("p (t s) -> p t s", s=SW))
                    ecc_tiles[j] = ecc
            if kind == "nrm" and tagn == 2:
                ed_nrm1 = ed

        # ---- phase C: interleaved two-strand DP ----
        # init alpha0 = ec_0 * minit  (A incl ghost, B)
        ecc0 = ecc_tiles[0]
        nc.vector.tensor_mul(TA[:, 0:WA], ecc0[:, SW:SW + WA], mit[:, 0:WA])
        nc.vector.tensor_mul(TB[:, 0:WB], ecc0[:, SW + SA:EW], mit[:, SA:SW])
        nr = 0
        for tau in range(1, th):
            ecc = ecc_tiles[tau // chk]
            off = (tau % chk) * EW
            # strand A: u, W, alpha' ; strand B interleaved
            nc.vector.tensor_add(TA[:, WA + 2:2 * WA + 2],
                                 TA[:, 0:WA], TA[:, 1:1 + WA])
            nc.vector.tensor_add(TB[:, WB + 2:2 * WB + 2],
                                 TB[:, 0:WB], TB[:, 1:1 + WB])
            ecv = ecc[:, off:off + EW].rearrange("p (h s) -> p h s", h=2)
            nc.vector.tensor_mul(
                WAt[:, :].rearrange("p (h s) -> p h s", h=2),
                TA[:, 2:2 * WA + 2].rearrange("p (h s) -> p h s", h=2),
                ecv[:, :, 0:WA])
            nc.vector.tensor_mul(
                WBt[:, :].rearrange("p (h s) -> p h s", h=2),
                TB[:, 2:2 * WB + 2].rearrange("p (h s) -> p h s", h=2),
                ecv[:, :, SA:SW])
            nc.vector.tensor_add(TA[:, 0:WA], WAt[:, 0:WA], WAt[:, WA:2 * WA])
            nc.vector.tensor_add(TB[:, 0:WB], WBt[:, 0:WB], WBt[:, WB:2 * WB])
            if tau % RESC == 0 and nr < nresc:
                # max over owned alpha; ghost refresh; rescale both strands
                nc.vector.reduce_max(histt[:, nr:nr + 1], TA[:, 2:SA], axis=AX)
                nc.vector.tensor_copy(TA[:, SA:WA], TB[:, 0:GH])
                nc.vector.reduce_max(hB[:, :], TB[:, 0:WB], axis=AX)
                nc.vector.tensor_max(histt[:, nr:nr + 1],
                                     histt[:, nr:nr + 1], hB[:, :])
                nc.vector.reciprocal_approx_fast(sclt[:, :], histt[:, nr:nr + 1])
                nc.vector.tensor_scalar(TB[:, 0:WB], TB[:, 0:WB],
                                        sclt[:, :], float(2.0 ** SETPOINT_LOG2),
                                        MULT, MULT)
                nc.vector.tensor_scalar(TA[:, 0:WA], TA[:, 0:WA],
                                        sclt[:, :], float(2.0 ** SETPOINT_LOG2),
                                        MULT, MULT)
                nr += 1

        # ---- final: assemble alpha, gamma on bwd rows, dump ----
        nc.vector.tensor_copy(alf[:, 0:SA], TA[:, 0:SA])
        nc.vector.tensor_copy(alf[:, SA:SW], TB[:, 0:WB])
        nc.vector.tensor_add(ut[:, :], alf[:, 0:SW], alf[:, 1:1 + SW])
        nc.vector.tensor_mul(vt[:, :], alf[:, 2:2 + SW], m2t[:, :])
        nc.vector.tensor_add(wt[:, :], ut[:, :], vt[:, :])
        nc.sync.dma_start(out=a_out.ap(), in_=alf[:, :])
        nc.sync.dma_start(out=g_out.ap(), in_=wt[:, :])
        nc.sync.dma_start(out=h_out.ap(), in_=histt[:, :])

    nc.compile()
    return nc


def kernel(y_true, y_pred):
    global _BUILT, _LAST_EXEC_NS, _LAST_RES
    from concourse.bass_utils import run_bass_kernel_spmd

    y_true = np.asarray(y_true)
    y_pred = np.ascontiguousarray(np.asarray(y_pred, dtype=np.float32))

    m2f, m2b, mif, mib, clsf, clsb = _host_metadata(y_true)

    if _BUILT is None:
        _BUILT = _build()
    nc = _BUILT

    import ml_dtypes
    bf = ml_dtypes.bfloat16
    ident = np.eye(128, dtype=np.float32)
    in_maps = []
    for c in range(NCORES):
        sl = slice(c * BN, (c + 1) * BN)
        m2 = np.concatenate([m2f[sl], m2b[sl]], axis=0)[:, ::-1].astype(bf)
        mi = np.concatenate([mif[sl], mib[sl]], axis=0)[:, ::-1].astype(bf)
        m2rep = np.tile(m2, (1, CHK))
        oh = np.zeros((C, BN * 2 * SW), np.float32)
        for e in range(BN):
            b = c * BN + e
            for dirn, cls in ((0, clsf[b]), (1, clsb[b])):
                colbase = (e * 2 + dirn) * SW
                rcls = cls[::-1]
                idx = np.nonzero(rcls >= 0)[0]
                oh[rcls[idx], colbase + idx] = 1.0
        in_maps.append({
            "ypred": y_pred[sl],
            "m2": np.ascontiguousarray(m2),
            "m2rep": np.ascontiguousarray(m2rep),
            "minit": np.ascontiguousarray(mi),
            "onehot": oh.astype(bf),
            "ident": ident.astype(bf),
        })

    import os
    trace = os.environ.get("CTC_TRACE", "") == "1"
    res = run_bass_kernel_spmd(nc, in_maps, list(range(NCORES)), trace=trace)
    _LAST_EXEC_NS = res.exec_time_ns
    _LAST_RES = res

    losses = np.zeros(B, np.float64)
    lng = np.log(np.float64(G))
    setlog = NRESC * SETPOINT_LOG2 * np.log(2.0)
    for c in range(NCORES):
        afin = res.results[c]["afin"].astype(np.float64)   # (128, SW+2) mirrored
        gfin = res.results[c]["gfin"].astype(np.float64)   # (128, SW) mirrored
        hist = res.results[c]["hist"].astype(np.float64)
        acc = np.log(np.maximum(hist[:, :NRESC], 1e-300)).sum(axis=1)
        afs = afin[:, 0:SW][:, ::-1]             # un-mirror -> natural order
        gfs = gfin[:, :][:, ::-1]
        af = afs[0:64, 0:S]                      # alpha_{T/2-1}[s]
        gm = gfs[64:128, 0:S][:, ::-1]           # W_{T/2-1}[s], u -> s
        P = (af * gm).sum(axis=1)
        lnP = np.log(np.maximum(P, 1e-300))
        losses[c * BN:(c + 1) * BN] = -(
            lnP + acc[:64] + acc[64:128] - 2 * setlog - T * lng)
    return np.float32(losses.mean())
